# revision 39
# baseline (speedup 1.0000x reference)
# DeepseekV2 MLA attention (T=2048, H=16) on 8 TRN2 NeuronCores.
#
# Two launches (host gather/transpose/normalize between them is free):
#   Launch 1 (seq x col 2D, 4 seq-blocks x 2 weight-col-halves): each core
#     computes raw low-rank latents for its 512-token block and weight-column
#     half.  RMSNorm (incl. sum-of-squares from the bf16 latents) + k_pe rope
#     are applied on the host (elementwise, cheap).
#   Launch 2 (head-parallel, 2 heads/core): up-projections, q-rope, causal
#     softmax attention (scores kept [k, q]; denominator via DVE column-sum
#     accumulation + one fp32 broadcast matmul), o_proj partial in bf16;
#     host sums the 8 partials.
#
# Perf notes (v2):
#   - DMA issue split across the SP + Act HWDGE rings and the gpsimd SWDGE
#     ring: each dma_start costs ~650ns of serialized issue time per engine
#   - warmup matmuls at kernel start bridge the ~7.4us runtime preamble +
#     first-load latency so the HAM clock-gate is warm when real work starts
#   - wqa/wkva packed into one DRAM tensor (fewer loads, >=2KB descriptors);
#     kva chain matmuls interleaved into the qa k-loop so the PE tracks the
#     incoming DMA stream instead of trailing it
#   - rope score matmuls (64-row stationary) issued as adjacent h0/h1 pairs:
#     base_partition 0/64 auto-derives tile_position (0,0)/(64,0) so the two
#     matmuls run CONCURRENTLY in disjoint row-groups of the PE array
#   - ki==0 exp written directly into the colsum tile (saves a DVE copy)
#   - o_proj/q-up stationary reuse; all o-chunk evacuations on DVE
import contextlib
import ctypes
import math
import sys
import types

import numpy as np

# ---------------------------------------------------------------- constants
H = 16
D_NOPE = 128
D_ROPE = 64
D_QK = D_NOPE + D_ROPE
D_V = 128
HID = 2048
Q_RANK = 1536
KV_RANK = 512
EPS = 1e-6
T = 2048
BASE = 10000.0
FACTOR = 40.0
ORIG_MAX = 4096
BETA_FAST = 32.0
BETA_SLOW = 1.0
MSCALE = 0.707
MSCALE_ALL = 0.707

N_CORES = 8
SEQB = 4                    # launch-1 sequence blocks
TC1 = T // SEQB             # 512 tokens per launch-1 core
WQH = Q_RANK // 2           # 768 wq_a columns per half
WKH = (KV_RANK + D_ROPE) // 2  # 288 wkv_a columns per half
WABH = WQH + WKH            # 1056 packed wq|wkv columns per half
HPC = H // N_CORES          # 2 heads per launch-2 core
QT = 512                    # q-tile width
KT = 128                    # k-tile height
RCH = Q_RANK // 128         # 12
KVCH = KV_RANK // 128       # 4
KCH = HID // 128            # 16


def _yarn_mscale(scale, mscale):
    return 1.0 if scale <= 1 else 0.1 * mscale * math.log(scale) + 1.0


SCALING = D_QK ** -0.5 * _yarn_mscale(FACTOR, MSCALE_ALL) ** 2

# ------------------------------------------------------- NTFF profiling shim
LAST_EXEC_NS = []


def _install_ntff_shim():
    try:
        import antenv.axon_hooks  # noqa: F401
        return
    except ImportError:
        pass
    try:
        so_path = "/opt/axon/libaxon_pjrt.so"
        lib = ctypes.CDLL(so_path)
        if not hasattr(lib, "axon_start_nrt_profile"):
            hook = None
        else:
            lib.axon_start_nrt_profile.argtypes = [
                ctypes.POINTER(ctypes.c_int64),
                ctypes.c_size_t,
            ]
            lib.axon_start_nrt_profile.restype = ctypes.c_int64
            lib.axon_stop_nrt_profile.argtypes = [ctypes.c_char_p]
            lib.axon_stop_nrt_profile.restype = ctypes.c_int64

            @contextlib.contextmanager
            def hook(output_dir, device_ids):
                import jax

                jax.devices()
                if device_ids:
                    ids = (ctypes.c_int64 * len(device_ids))(*device_ids)
                    rc = lib.axon_start_nrt_profile(ids, len(device_ids))
                else:
                    rc = lib.axon_start_nrt_profile(None, 0)
                if rc != 0:
                    raise RuntimeError(f"axon_start_nrt_profile rc={rc}")
                try:
                    yield
                finally:
                    n = lib.axon_stop_nrt_profile(str(output_dir).encode())
                    if n < 0:
                        raise RuntimeError(f"axon_stop_nrt_profile rc={n}")

        mod = types.ModuleType("antenv.axon_hooks")
        mod.get_axon_ntff_profile_hook = lambda: hook
        mod.set_axon_ntff_profile_hook = lambda h: None
        sys.modules["antenv.axon_hooks"] = mod
    except Exception:
        pass


_install_ntff_shim()

# ------------------------------------------------------------- host helpers


def _rope_tables(positions):
    dim = D_ROPE
    pos_freqs = BASE ** (np.arange(0, dim, 2, dtype=np.float64) / dim)
    inv_extra = 1.0 / pos_freqs
    inv_inter = 1.0 / (FACTOR * pos_freqs)

    def corr(nr):
        return dim * math.log(ORIG_MAX / (nr * 2 * math.pi)) / (2 * math.log(BASE))

    low = max(math.floor(corr(BETA_FAST)), 0)
    high = min(math.ceil(corr(BETA_SLOW)), dim - 1)
    ramp = np.clip(
        (np.arange(dim // 2, dtype=np.float64) - low) / max(high - low, 0.001), 0.0, 1.0
    )
    mask = 1.0 - ramp
    inv_freq = inv_inter * (1.0 - mask) + inv_extra * mask
    freqs = np.outer(np.asarray(positions, np.float64), inv_freq)
    m = _yarn_mscale(FACTOR, MSCALE) / _yarn_mscale(FACTOR, MSCALE_ALL)
    return (np.cos(freqs) * m).astype(np.float32), (np.sin(freqs) * m).astype(np.float32)


# ------------------------------------------------------------ bass builders
_BUILD_CACHE = {}


def _build_phase1():
    from concourse import bacc, mybir
    from concourse.tile import TileContext

    F32 = mybir.dt.float32
    BF16 = mybir.dt.bfloat16

    nc = bacc.Bacc()
    hTp = nc.dram_tensor("hTp", [128, KCH, TC1], BF16, kind="ExternalInput")
    wabp = nc.dram_tensor("wabp", [128, KCH, WABH], BF16, kind="ExternalInput")
    ab_out = nc.dram_tensor("ab", [128, TC1 // 128, WABH], BF16,
                            kind="ExternalOutput")

    MT = TC1 // 128  # 4 token tiles

    with TileContext(nc) as tc, contextlib.ExitStack() as ctx:
        pool = ctx.enter_context(tc.tile_pool(name="sb", bufs=1))

        hT_sb = pool.tile([128, KCH, TC1], BF16, tag="hT")
        wab_sb = pool.tile([128, KCH, WABH], BF16, tag="wab")
        ab_st = pool.tile([128, MT, WABH], BF16, tag="abst")
        warm_sb = pool.tile([128, 512], BF16, tag="warm")
        nc.vector.memset(warm_sb[:, :], 0.25)

        # warmup matmuls: no data deps, keep the PE busy through the DMA fill
        with tc.tile_pool(name="ppw", bufs=1, space="PSUM") as ppw:
            w_ps = ppw.tile([128, 512], F32, tag="w")
            for i in range(5):
                nc.tensor.matmul(w_ps[:, :], warm_sb[:, 0:128], warm_sb[:, :],
                                 start=True, stop=True)

        # loads: single ring, in consumption order; only 8 HWDGE completion
        # semaphores exist, so stay at <=8 outstanding dma_starts
        ksplits = [(0, 1), (1, 5), (5, 10), (10, 16)]
        for a, b in ksplits:
            nc.sync.dma_start(out=hT_sb[:, a:b, :], in_=hTp[:, a:b, :])
            nc.sync.dma_start(out=wab_sb[:, a:b, :], in_=wabp[:, a:b, :])

        # qa + kva chains interleaved per k: the static scheduler cannot fill
        # runtime DMA waits, so each k-chunk's full PE work is issued
        # together to ride the stream densely
        with tc.tile_pool(name="ppq", bufs=1, space="PSUM") as ppq, \
             tc.tile_pool(name="ppk", bufs=2, space="PSUM") as ppk:
            for mg in range(2):
                qa_ps = [ppq.tile([128, WQH], F32, tag=f"qa{mi}",
                                  name=f"qa{mg}_{mi}") for mi in range(2)]
                kv_ps = [ppk.tile([128, WKH], F32, tag=f"kv{mi}",
                                  name=f"kv{mg}_{mi}") for mi in range(2)]
                for k in range(KCH):
                    st = (k == 0)
                    sp = (k == KCH - 1)
                    for mi in range(2):
                        m = 2 * mg + mi
                        stat = hT_sb[:, k, m * 128:(m + 1) * 128]
                        nc.tensor.matmul(qa_ps[mi][:, 0:512], stat,
                                         wab_sb[:, k, 0:512],
                                         start=st, stop=sp)
                        nc.tensor.matmul(qa_ps[mi][:, 512:WQH], stat,
                                         wab_sb[:, k, 512:WQH],
                                         start=st, stop=sp)
                        nc.tensor.matmul(kv_ps[mi][:, :], stat,
                                         wab_sb[:, k, WQH:WABH],
                                         start=st, stop=sp)
                for mi in range(2):
                    m = 2 * mg + mi
                    if mi == 0:
                        nc.vector.tensor_copy(ab_st[:, m, 0:WQH],
                                              qa_ps[mi][:, :])
                        nc.scalar.copy(ab_st[:, m, WQH:WABH], kv_ps[mi][:, :])
                    else:
                        nc.scalar.copy(ab_st[:, m, 0:WQH], qa_ps[mi][:, :])
                        nc.vector.tensor_copy(ab_st[:, m, WQH:WABH],
                                              kv_ps[mi][:, :])
                eng = nc.sync if mg == 0 else nc.scalar
                eng.dma_start(out=ab_out[:, 2 * mg:2 * mg + 2, :],
                              in_=ab_st[:, 2 * mg:2 * mg + 2, :])

    nc.finalize()
    return nc


def _build_phase2():
    from concourse import bacc, mybir
    from concourse.tile import TileContext

    F32 = mybir.dt.float32
    BF16 = mybir.dt.bfloat16
    FP16 = mybir.dt.float16
    AF = mybir.ActivationFunctionType
    OP = mybir.AluOpType
    EXPB = -8.0 * math.log(2.0)  # exp bias; cancels in softmax, keeps fp16 range

    nc = bacc.Bacc()
    kvaTp = nc.dram_tensor("kvaTp", [128, KVCH, T], BF16, kind="ExternalInput")
    qaTp = nc.dram_tensor("qaTp", [128, RCH, T], BF16, kind="ExternalInput")
    kpe2 = nc.dram_tensor("kpe2", [128, T], BF16, kind="ExternalInput")
    wkbn = nc.dram_tensor("wkbn", [128, KVCH, HPC, 128], BF16, kind="ExternalInput")
    wkbv = nc.dram_tensor("wkbv", [128, KVCH, 256], BF16, kind="ExternalInput")
    wq3 = nc.dram_tensor("wq3", [128, RCH, 3, 128], BF16, kind="ExternalInput")
    wop = nc.dram_tensor("wop", [128, HPC, HID], BF16, kind="ExternalInput")
    csd = nc.dram_tensor("csd", [128, T], BF16, kind="ExternalInput")
    ssd = nc.dram_tensor("ssd", [128, T], BF16, kind="ExternalInput")
    maskd = nc.dram_tensor("maskd", [128, 896], FP16, kind="ExternalInput")
    out_p = nc.dram_tensor("out_p", [T, HID], BF16, kind="ExternalOutput")

    with TileContext(nc) as tc, contextlib.ExitStack() as ctx:
        persist = ctx.enter_context(tc.tile_pool(name="persist", bufs=1))

        kvaT_sb = persist.tile([128, KVCH, T], BF16, tag="kvaT")
        qaT_sb = persist.tile([128, RCH, T], BF16, tag="qaT")
        kpe2_sb = persist.tile([128, T], BF16, tag="kpe2")
        wkbn_sb = persist.tile([128, KVCH, HPC, 128], BF16, tag="wkbn")
        wkbv_sb = persist.tile([128, KVCH, 256], BF16, tag="wkbv")
        wq3_sb = persist.tile([128, RCH, 3, 128], BF16, tag="wq3")
        wo_sb = persist.tile([128, HPC, HID], BF16, tag="wo")
        cs_sb = persist.tile([128, T], BF16, tag="cs")
        ss_sb = persist.tile([128, T], BF16, tag="ss")
        mask_sb = persist.tile([128, 896], FP16, tag="mask")
        ones_sb = persist.tile([128, 128], FP16, tag="ones")
        nc.vector.memset(ones_sb[:, :], 1.0)
        expb_sb = persist.tile([128, 1], F32, tag="expb")
        nc.vector.memset(expb_sb[:, :], EXPB)
        warm_sb = persist.tile([128, 512], BF16, tag="warm")
        nc.vector.memset(warm_sb[:, :], 0.25)
        gate_sb = persist.tile([128, 2], BF16, tag="gate")
        # touch Exp on the ACT engine now so its table load lands in the
        # preamble shadow instead of blocking the q-up evacuation later
        actw_sb = persist.tile([128, 1], F32, tag="actw")
        nc.scalar.activation(out=actw_sb[:, :], in_=expb_sb[:, :],
                             func=AF.Exp, scale=1.0, bias=expb_sb[:, :])

        knopeT = [persist.tile([128, T], BF16, tag=f"knopeT{h}", name=f"knopeT{h}") for h in range(HPC)]
        v_nat = [persist.tile([128, T], FP16, tag=f"vnat{h}", name=f"vnat{h}") for h in range(HPC)]
        qnT = [persist.tile([128, T], BF16, tag=f"qnT{h}", name=f"qnT{h}") for h in range(HPC)]
        qpeT = persist.tile([128, T], BF16, tag="qpeT")  # [h0 x'|y' ; h1 x'|y']
        aoT = [persist.tile([128, T], BF16, tag=f"aoT{h}", name=f"aoT{h}") for h in range(HPC)]
        colsum = [persist.tile([128, QT], FP16, tag=f"colsum{h}", name=f"colsum{h}") for h in range(HPC)]

        # warmup matmuls bridge the runtime preamble + first-load latency
        with tc.tile_pool(name="ppw", bufs=1, space="PSUM") as ppw:
            w_ps = ppw.tile([128, 512], F32, tag="w")
            for i in range(9):
                nc.tensor.matmul(w_ps[:, :], warm_sb[:, 0:128], warm_sb[:, :],
                                 start=True, stop=True)

        # loads, split across the three DMA-issue engines:
        #  SP ring: PE-critical order (stage A/B inputs), streams alone first
        #  Act ring: the big qaT stream, gated behind the critical kva loads
        #  gpsimd SWDGE: late attention inputs, same gate
        # single HWDGE ring, strict consumption order: ring order IS the
        # execution order, which encodes criticality without gate hacks; big
        # consolidated loads keep the 8 completion semaphores from running
        # out (they recycle as the consuming matmuls execute)
        nc.sync.dma_start(out=wkbn_sb[:, :, :, :], in_=wkbn[:, :, :, :])
        nc.sync.dma_start(out=kvaT_sb[:, 0:1, :], in_=kvaTp[:, 0:1, :])
        nc.sync.dma_start(out=kvaT_sb[:, 1:2, :], in_=kvaTp[:, 1:2, :])
        nc.sync.dma_start(out=kvaT_sb[:, 2:4, :], in_=kvaTp[:, 2:4, :])
        nc.sync.dma_start(out=wq3_sb[:, 0:6, :, :], in_=wq3[:, 0:6, :, :])
        nc.sync.dma_start(out=qaT_sb[:, 0:2, :], in_=qaTp[:, 0:2, :])
        nc.sync.dma_start(out=wq3_sb[:, 6:12, :, :], in_=wq3[:, 6:12, :, :])
        nc.sync.dma_start(out=qaT_sb[:, 2:5, :], in_=qaTp[:, 2:5, :])
        nc.sync.dma_start(out=wkbv_sb[:, :, :], in_=wkbv[:, :, :])
        nc.sync.dma_start(out=qaT_sb[:, 5:8, :], in_=qaTp[:, 5:8, :])
        nc.sync.dma_start(out=cs_sb[:, :], in_=csd[:, :])
        nc.sync.dma_start(out=ss_sb[:, :], in_=ssd[:, :])
        nc.sync.dma_start(out=qaT_sb[:, 8:12, :], in_=qaTp[:, 8:12, :])
        nc.sync.dma_start(out=kpe2_sb[:, :], in_=kpe2[:, :])
        nc.sync.dma_start(out=mask_sb[:, :], in_=maskd[:, :])
        nc.sync.dma_start(out=wo_sb[:, :, :], in_=wop[:, :, :])

        # ---------------- stage 1a: k_nope^T = wkbn^T kva, k-outer ----------
        with tc.tile_pool(name="ppkn", bufs=1, space="PSUM") as ppkn:
            kn_ps = [[ppkn.tile([128, 512], F32, tag=f"kn{h}_{n}", name=f"kn{h}_{n}")
                      for n in range(4)] for h in range(HPC)]
            for k in range(KVCH):
                for h in range(HPC):
                    for n in range(4):
                        nsl = slice(n * 512, (n + 1) * 512)
                        nc.tensor.matmul(
                            kn_ps[h][n][:, :], wkbn_sb[:, k, h, :],
                            kvaT_sb[:, k, nsl],
                            start=(k == 0), stop=(k == KVCH - 1))
            for h in range(HPC):
                for n in range(4):
                    nsl = slice(n * 512, (n + 1) * 512)
                    if n % 2 == 0:
                        nc.scalar.copy(knopeT[h][:, nsl], kn_ps[h][n][:, :])
                    else:
                        nc.vector.tensor_copy(knopeT[h][:, nsl],
                                              kn_ps[h][n][:, :])

        # ------------- stage 2: q up-projections + rope (2-qtr groups);
        # v up-projection shares the PSUM scope (disjoint banks) so there is
        # no pool barrier before it and it bridges the last rope tail
        with tc.tile_pool(name="ppg", bufs=2, space="PSUM") as ppg, \
             tc.tile_pool(name="ppv", bufs=2, space="PSUM") as ppv, \
             tc.tile_pool(name="qwork", bufs=2) as qwork:
            for qg in range(2):
                qtrs = [2 * qg, 2 * qg + 1]
                qn_ps = {}
                qp_ps = {}
                for qtr in qtrs:
                    qn_ps[qtr] = [ppg.tile([128, QT], F32, tag=f"qn{h}",
                                           name=f"qn{h}_{qtr}")
                                  for h in range(HPC)]
                    qp_ps[qtr] = ppg.tile([128, QT], F32, tag="qp",
                                          name=f"qp_{qtr}")
                for k in range(RCH):
                    st = (k == 0)
                    sp = (k == RCH - 1)
                    # stationary-reuse order: each stationary serves both qtrs
                    for h in range(HPC):
                        for qtr in qtrs:
                            qsl = slice(qtr * QT, (qtr + 1) * QT)
                            nc.tensor.matmul(qn_ps[qtr][h][:, :],
                                             wq3_sb[:, k, h, :],
                                             qaT_sb[:, k, qsl],
                                             start=st, stop=sp)
                    for qtr in qtrs:
                        qsl = slice(qtr * QT, (qtr + 1) * QT)
                        nc.tensor.matmul(qp_ps[qtr][:, :], wq3_sb[:, k, 2, :],
                                         qaT_sb[:, k, qsl],
                                         start=st, stop=sp)
                for qtr in qtrs:
                    qsl = slice(qtr * QT, (qtr + 1) * QT)
                    # rope first: it is the longest chain (copy -> swaps ->
                    # DVE muls) gating the attention start; qnT copies follow
                    qp_sb = qwork.tile([128, QT], F32, tag="qpsb",
                                       name=f"qpsb{qtr}")
                    nc.scalar.copy(qp_sb[:, :], qp_ps[qtr][:, :])
                    sw_sb = qwork.tile([128, QT], F32, tag="swsb",
                                       name=f"swsb{qtr}")
                    nc.gpsimd.dma_start(out=sw_sb[0:32, :], in_=qp_sb[32:64, :])
                    nc.gpsimd.dma_start(out=sw_sb[32:64, :], in_=qp_sb[0:32, :])
                    nc.gpsimd.dma_start(out=sw_sb[64:96, :], in_=qp_sb[96:128, :])
                    nc.gpsimd.dma_start(out=sw_sb[96:128, :], in_=qp_sb[64:96, :])
                    ta = qwork.tile([128, QT], BF16, tag="ta", name=f"ta{qtr}")
                    nc.vector.tensor_tensor(ta[:, :], qp_sb[:, :], cs_sb[:, qsl],
                                            op=OP.mult)
                    tb = qwork.tile([128, QT], BF16, tag="tb", name=f"tb{qtr}")
                    nc.vector.tensor_tensor(tb[:, :], sw_sb[:, :], ss_sb[:, qsl],
                                            op=OP.mult)
                    nc.vector.tensor_tensor(qpeT[:, qsl], ta[:, :], tb[:, :],
                                            op=OP.add)
                    nc.scalar.copy(qnT[0][:, qsl], qn_ps[qtr][0][:, :])
                    nc.vector.tensor_copy(qnT[1][:, qsl], qn_ps[qtr][1][:, :])

            # v up-projection: real PE work independent of the rope tables
            for t in range(T // 128):
                tsl = slice(t * 128, (t + 1) * 128)
                v_ps = ppv.tile([128, 256], F32, tag="v", name=f"v{t}")
                for k in range(KVCH):
                    nc.tensor.matmul(v_ps[:, :], kvaT_sb[:, k, tsl],
                                     wkbv_sb[:, k, :],
                                     start=(k == 0), stop=(k == KVCH - 1))
                if t % 2 == 0:
                    nc.scalar.copy(v_nat[0][:, tsl], v_ps[:, 0:128])
                    nc.vector.tensor_copy(v_nat[1][:, tsl], v_ps[:, 128:256])
                else:
                    nc.vector.tensor_copy(v_nat[0][:, tsl], v_ps[:, 0:128])
                    nc.scalar.copy(v_nat[1][:, tsl], v_ps[:, 128:256])

        # ------- stage 3: attention; softmax tail + o_proj pipelined 1 qtr ----
        # both heads of a k-tile share one [128, 2, 512] PSUM pair so the
        # exp / mask / colsum work runs as ONE wide instruction per k-tile
        # (instruction overhead halved) and the four score matmuls become
        # ready together -> the two 64-row rope matmuls issue adjacently and
        # run concurrently in disjoint row-groups of the PE array
        with tc.tile_pool(name="pps", bufs=2, space="PSUM") as pps, \
             tc.tile_pool(name="ppu", bufs=2, space="PSUM") as ppu, \
             tc.tile_pool(name="ppo", bufs=2, space="PSUM") as ppo, \
             tc.tile_pool(name="awork", bufs=8) as awork, \
             tc.tile_pool(name="rwork", bufs=2) as rwork, \
             tc.tile_pool(name="owork", bufs=3) as owork:

            o_sb_map = {}
            o_count = [0]

            def emit_o_unit(qtr, tt, j):
                # one o_proj 512-col chunk for token tile tt of q-range qtr
                q0o = qtr * QT
                tslo = slice(q0o + tt * 128, q0o + (tt + 1) * 128)
                jsl = slice(j * 512, (j + 1) * 512)
                if j == 0:
                    o_sb_map[(qtr, tt)] = owork.tile(
                        [128, HID], BF16, tag="osb", name=f"o{qtr}_{tt}")
                o_sb = o_sb_map[(qtr, tt)]
                o_ps = ppo.tile([128, 512], F32, tag="o",
                                name=f"op{qtr}_{tt}_{j}")
                for h in range(HPC):
                    nc.tensor.matmul(o_ps[:, :], aoT[h][:, tslo],
                                     wo_sb[:, h, jsl],
                                     start=(h == 0), stop=(h == HPC - 1))
                o_count[0] += 1
                if o_count[0] % 2 == 0:
                    nc.vector.tensor_copy(o_sb[:, jsl], o_ps[:, :])
                else:
                    nc.scalar.copy(o_sb[:, jsl], o_ps[:, :])
                if j == 3:
                    nc.sync.dma_start(out=out_p[tslo, :], in_=o_sb[:, :])

            def make_tail(qtr, h, un_t):
                def emit():
                    qslh = slice(qtr * QT, (qtr + 1) * QT)
                    denb_ps = ppo.tile([128, QT], F32, tag="o",
                                       name=f"db{h}_{qtr}")
                    nc.tensor.matmul(denb_ps[:, :], ones_sb[:, :],
                                     colsum[h][:, :], start=True, stop=True,
                                     skip_group_check=True)
                    recip = rwork.tile([128, QT], F32, tag="recip",
                                       name=f"r{h}_{qtr}")
                    nc.vector.reciprocal_approx_fast(out=recip[:, :],
                                                     in_=denb_ps[:, :])
                    nc.vector.tensor_tensor(aoT[h][:, qslh], un_t[h][:, :],
                                            recip[:, :], op=OP.mult)
                return emit

            filler = []
            # causally independent q-ranges; end on the shortest (qtr 0) so
            # the previous qtr's deferred tail drains inside a loop, leaving
            # the smallest possible serial flush
            for qtr in (1, 2, 3, 0):
                q0 = qtr * QT
                n_k = (q0 + QT) // KT
                # softmax tails of the previous qtr run first: they free the
                # un tiles this qtr's AV accumulation needs
                for _ in range(2):
                    if filler:
                        filler.pop(0)()
                un_ps = [ppu.tile([128, QT], F32, tag="un",
                                  name=f"un{h}_{qtr}") for h in range(HPC)]
                for ki in range(n_k):
                    k0 = ki * KT
                    ksl = slice(k0, k0 + KT)
                    d = k0 - q0
                    coff = max(d, 0)
                    w = QT - coff
                    msl = slice(q0 + coff, q0 + QT)
                    if filler:
                        filler.pop(0)()
                    # both heads' scores share one [128, 2, 512] PSUM pair:
                    # the four matmuls become ready together, so the two
                    # 64-row rope matmuls issue adjacently and run
                    # concurrently in disjoint row-groups of the PE array
                    s2 = pps.tile([128, HPC, QT], F32, tag="s",
                                  name=f"s_{qtr}_{ki}")
                    for h in range(HPC):
                        nc.tensor.matmul(s2[:, h, 0:w], knopeT[h][:, ksl],
                                         qnT[h][:, msl], start=True, stop=False,
                                         skip_group_check=True)
                    for h in range(HPC):
                        hb = slice(64 * h, 64 * h + 64)
                        nc.tensor.matmul(s2[:, h, 0:w], kpe2_sb[hb, ksl],
                                         qpeT[hb, msl], start=False, stop=True,
                                         skip_group_check=True)
                    if filler:
                        filler.pop(0)()
                    # per-head softmax plumbing on contiguous 2D slices
                    # (strided 3D APs run far below peak on ACT/DVE)
                    for h in range(HPC):
                        if ki == 0:
                            # exp straight into the colsum tile: saves a copy
                            nc.scalar.activation(out=colsum[h][:, 0:w],
                                                 in_=s2[:, h, 0:w],
                                                 func=AF.Exp, scale=SCALING,
                                                 bias=expb_sb[:, :])
                            if d >= 0:
                                nc.vector.tensor_tensor(
                                    colsum[h][:, 0:w], colsum[h][:, 0:w],
                                    mask_sb[:, 384:384 + w], op=OP.mult)
                            mov = colsum[h]
                        else:
                            expT = awork.tile([128, QT], FP16, tag="expT",
                                              name=f"e{h}_{qtr}_{ki}")
                            nc.scalar.activation(out=expT[:, 0:w],
                                                 in_=s2[:, h, 0:w],
                                                 func=AF.Exp, scale=SCALING,
                                                 bias=expb_sb[:, :])
                            if d >= 0:
                                nc.vector.tensor_tensor(
                                    expT[:, 0:w], expT[:, 0:w],
                                    mask_sb[:, 384:384 + w], op=OP.mult)
                            nc.vector.tensor_tensor(
                                colsum[h][:, coff:QT], colsum[h][:, coff:QT],
                                expT[:, 0:w], op=OP.add)
                            mov = expT
                        nc.tensor.matmul(un_ps[h][:, coff:QT],
                                         v_nat[h][:, ksl], mov[:, 0:w],
                                         start=(ki == 0), stop=(ki == n_k - 1),
                                         skip_group_check=True)
                while filler:
                    filler.pop(0)()
                filler = [make_tail(qtr, h, un_ps) for h in range(HPC)]
                filler += [(lambda a, b, c: (lambda: emit_o_unit(a, b, c)))(
                    qtr, tt, j) for tt in range(QT // 128) for j in range(4)]
            while filler:
                filler.pop(0)()

    nc.finalize()
    return nc


def _get_built(name):
    if name not in _BUILD_CACHE:
        _BUILD_CACHE[name] = _build_phase1() if name == "p1" else _build_phase2()
    return _BUILD_CACHE[name]


# ---------------------------------------------------------------- kernel()


def kernel(positions, hidden_states, wq_a, q_a_norm_w, wq_b, wkv_a, kv_a_norm_w,
           wkv_b, wo):
    import os

    from concourse.bass_utils import run_bass_kernel_spmd
    import ml_dtypes

    BFNP = ml_dtypes.bfloat16
    trace = bool(os.environ.get("BASS_KERNEL_TRACE"))
    LAST_EXEC_NS.clear()

    positions = np.asarray(positions)
    hidden = np.asarray(hidden_states, np.float32)
    wq_a = np.asarray(wq_a, np.float32)
    wq_b = np.asarray(wq_b, np.float32)
    wkv_a = np.asarray(wkv_a, np.float32)
    wkv_b = np.asarray(wkv_b, np.float32)
    wo = np.asarray(wo, np.float32)
    q_a_norm_w = np.asarray(q_a_norm_w, np.float32)
    kv_a_norm_w = np.asarray(kv_a_norm_w, np.float32)

    cos, sin = _rope_tables(positions)  # [T, 32] f32

    # ---------------- launch 1: latents (4 seq blocks x 2 col halves) -------
    hidden_bf = hidden.astype(BFNP)
    wab_halves = []
    for half in range(2):
        wq_h = wq_a[:, half * WQH:(half + 1) * WQH]
        wk_h = wkv_a[:, half * WKH:(half + 1) * WKH]
        wab = np.concatenate([wq_h, wk_h], axis=1).astype(BFNP)  # [2048, 1056]
        wab_halves.append(np.ascontiguousarray(
            wab.reshape(KCH, 128, WABH).transpose(1, 0, 2)))

    in_maps1 = []
    for c in range(N_CORES):
        seq, half = c // 2, c % 2
        hs = hidden_bf[seq * TC1:(seq + 1) * TC1]  # [512, 2048]
        hTp = np.ascontiguousarray(hs.reshape(TC1, KCH, 128).transpose(2, 1, 0))
        in_maps1.append({
            "hTp": hTp,
            "wabp": wab_halves[half],
        })

    nc1 = _get_built("p1")
    res1 = run_bass_kernel_spmd(nc1, in_maps1, core_ids=list(range(N_CORES)),
                                trace=trace)
    if trace:
        LAST_EXEC_NS.append(res1.exec_time_ns)

    qa_full = np.empty((T, Q_RANK), np.float32)
    kva_full = np.empty((T, KV_RANK + D_ROPE), np.float32)
    for c in range(N_CORES):
        seq, half = c // 2, c % 2
        r = res1.results[c]
        tsl = slice(seq * TC1, (seq + 1) * TC1)
        ab = np.asarray(r["ab"], np.float32).transpose(1, 0, 2).reshape(
            TC1, WABH)
        qa_full[tsl, half * WQH:(half + 1) * WQH] = ab[:, 0:WQH]
        kva_full[tsl, half * WKH:(half + 1) * WKH] = ab[:, WQH:WABH]

    # host RMSNorm (ssq from the bf16 latents; negligible vs fp32)
    rstd_q = 1.0 / np.sqrt((qa_full ** 2).mean(1) + EPS)
    rstd_kv = 1.0 / np.sqrt((kva_full[:, :KV_RANK] ** 2).mean(1) + EPS)

    qa_n = qa_full * rstd_q[:, None]
    kva_n = kva_full[:, :KV_RANK] * rstd_kv[:, None]
    kpe_raw = kva_full[:, KV_RANK:]  # [T, 64], not normalized

    # host rope for k_pe (shared across heads); rows de-interleaved evens|odds
    x1, x2 = kpe_raw[:, 0::2], kpe_raw[:, 1::2]
    kx = x1 * cos - x2 * sin
    ky = x2 * cos + x1 * sin
    kpeT = np.concatenate([kx.T, ky.T], 0)          # [64, T]
    kpe2_host = np.ascontiguousarray(
        np.concatenate([kpeT, kpeT], 0).astype(BFNP))  # [128, T]

    qaT_p = np.ascontiguousarray(
        qa_n.T.reshape(RCH, 128, T).transpose(1, 0, 2).astype(BFNP))
    kvaT_p = np.ascontiguousarray(
        kva_n.T.reshape(KVCH, 128, T).transpose(1, 0, 2).astype(BFNP))

    # q-rope tables: CS = cos tiled 4x, SS = [-s, +s, -s, +s]
    cos_t = cos.T  # [32, T]
    sin_t = sin.T
    cs_host = np.ascontiguousarray(
        np.concatenate([cos_t] * 4, 0).astype(BFNP))
    ss_host = np.ascontiguousarray(
        np.concatenate([-sin_t, sin_t, -sin_t, sin_t], 0).astype(BFNP))

    cols = np.arange(896) - 384
    bigmask = np.ascontiguousarray(
        (cols[None, :] >= np.arange(128)[:, None]).astype(np.float16))

    # fold RMSNorm weights into the up-projection weights
    wq_b_eff = (wq_b * q_a_norm_w[:, None]).reshape(Q_RANK, H, D_QK)
    wkv_b_eff = (wkv_b * kv_a_norm_w[:, None]).reshape(KV_RANK, H, D_NOPE + D_V)
    wo_r = wo.reshape(H, D_V, HID)

    def pack_stat(w):  # [K*128, M] -> [128, K, M]
        kch = w.shape[0] // 128
        return np.ascontiguousarray(
            w.reshape(kch, 128, w.shape[1]).transpose(1, 0, 2).astype(BFNP))

    # ---------------- launch 2 ----------------
    nc2 = _get_built("p2")
    in_maps2 = []
    for c in range(N_CORES):
        heads = [c * HPC, c * HPC + 1]
        wqbn_w = wq_b_eff[:, heads, :D_NOPE].reshape(Q_RANK, HPC * 128)
        pe = wq_b_eff[:, heads, D_NOPE:]            # [R, 2, 64]
        px, py = pe[:, :, 0::2], pe[:, :, 1::2]     # [R, 2, 32]
        wqbp_w = np.concatenate(
            [px[:, 0], py[:, 0], px[:, 1], py[:, 1]], 1)  # [R, 128] order C
        wq3_w = np.concatenate([wqbn_w, wqbp_w], 1)  # [R, 384]
        wkbn_w = wkv_b_eff[:, heads, :D_NOPE].reshape(KV_RANK, HPC * 128)
        wkbv_w = wkv_b_eff[:, heads, D_NOPE:].reshape(KV_RANK, HPC * 128)
        in_maps2.append({
            "kvaTp": kvaT_p,
            "qaTp": qaT_p,
            "kpe2": kpe2_host,
            "wkbn": np.ascontiguousarray(
                pack_stat(wkbn_w).reshape(128, KVCH, HPC, 128)),
            "wkbv": pack_stat(wkbv_w),
            "wq3": np.ascontiguousarray(
                pack_stat(wq3_w).reshape(128, RCH, 3, 128)),
            "wop": np.ascontiguousarray(
                wo_r[heads].transpose(1, 0, 2).astype(BFNP)),
            "csd": cs_host,
            "ssd": ss_host,
            "maskd": bigmask,
        })
    res2 = run_bass_kernel_spmd(nc2, in_maps2, core_ids=list(range(N_CORES)),
                                trace=trace)
    if trace:
        LAST_EXEC_NS.append(res2.exec_time_ns)

    out = np.zeros((T, HID), np.float64)
    for c in range(N_CORES):
        out += res2.results[c]["out_p"].astype(np.float64)
    return out.astype(np.float32)


# revision 40
# speedup vs baseline: 1.0108x; 1.0108x over previous
# DeepseekV2 MLA attention (T=2048, H=16) on 8 TRN2 NeuronCores.
#
# Two launches (host gather/transpose/normalize between them is free):
#   Launch 1 (seq x col 2D, 4 seq-blocks x 2 weight-col-halves): each core
#     computes raw low-rank latents for its 512-token block and weight-column
#     half.  RMSNorm (incl. sum-of-squares from the bf16 latents) + k_pe rope
#     are applied on the host (elementwise, cheap).
#   Launch 2 (head-parallel, 2 heads/core): up-projections, q-rope, causal
#     softmax attention (scores kept [k, q]; denominator via DVE column-sum
#     accumulation + one fp32 broadcast matmul), o_proj partial in bf16;
#     host sums the 8 partials.
#
# Perf notes (v2):
#   - DMA issue split across the SP + Act HWDGE rings and the gpsimd SWDGE
#     ring: each dma_start costs ~650ns of serialized issue time per engine
#   - warmup matmuls at kernel start bridge the ~7.4us runtime preamble +
#     first-load latency so the HAM clock-gate is warm when real work starts
#   - wqa/wkva packed into one DRAM tensor (fewer loads, >=2KB descriptors);
#     kva chain matmuls interleaved into the qa k-loop so the PE tracks the
#     incoming DMA stream instead of trailing it
#   - rope score matmuls (64-row stationary) issued as adjacent h0/h1 pairs:
#     base_partition 0/64 auto-derives tile_position (0,0)/(64,0) so the two
#     matmuls run CONCURRENTLY in disjoint row-groups of the PE array
#   - ki==0 exp written directly into the colsum tile (saves a DVE copy)
#   - o_proj/q-up stationary reuse; all o-chunk evacuations on DVE
import contextlib
import ctypes
import math
import sys
import types

import numpy as np

# ---------------------------------------------------------------- constants
H = 16
D_NOPE = 128
D_ROPE = 64
D_QK = D_NOPE + D_ROPE
D_V = 128
HID = 2048
Q_RANK = 1536
KV_RANK = 512
EPS = 1e-6
T = 2048
BASE = 10000.0
FACTOR = 40.0
ORIG_MAX = 4096
BETA_FAST = 32.0
BETA_SLOW = 1.0
MSCALE = 0.707
MSCALE_ALL = 0.707

N_CORES = 8
SEQB = 4                    # launch-1 sequence blocks
TC1 = T // SEQB             # 512 tokens per launch-1 core
WQH = Q_RANK // 2           # 768 wq_a columns per half
WKH = (KV_RANK + D_ROPE) // 2  # 288 wkv_a columns per half
WABH = WQH + WKH            # 1056 packed wq|wkv columns per half
HPC = H // N_CORES          # 2 heads per launch-2 core
QT = 512                    # q-tile width
KT = 128                    # k-tile height
RCH = Q_RANK // 128         # 12
KVCH = KV_RANK // 128       # 4
KCH = HID // 128            # 16


def _yarn_mscale(scale, mscale):
    return 1.0 if scale <= 1 else 0.1 * mscale * math.log(scale) + 1.0


SCALING = D_QK ** -0.5 * _yarn_mscale(FACTOR, MSCALE_ALL) ** 2

# ------------------------------------------------------- NTFF profiling shim
LAST_EXEC_NS = []


def _install_ntff_shim():
    try:
        import antenv.axon_hooks  # noqa: F401
        return
    except ImportError:
        pass
    try:
        so_path = "/opt/axon/libaxon_pjrt.so"
        lib = ctypes.CDLL(so_path)
        if not hasattr(lib, "axon_start_nrt_profile"):
            hook = None
        else:
            lib.axon_start_nrt_profile.argtypes = [
                ctypes.POINTER(ctypes.c_int64),
                ctypes.c_size_t,
            ]
            lib.axon_start_nrt_profile.restype = ctypes.c_int64
            lib.axon_stop_nrt_profile.argtypes = [ctypes.c_char_p]
            lib.axon_stop_nrt_profile.restype = ctypes.c_int64

            @contextlib.contextmanager
            def hook(output_dir, device_ids):
                import jax

                jax.devices()
                if device_ids:
                    ids = (ctypes.c_int64 * len(device_ids))(*device_ids)
                    rc = lib.axon_start_nrt_profile(ids, len(device_ids))
                else:
                    rc = lib.axon_start_nrt_profile(None, 0)
                if rc != 0:
                    raise RuntimeError(f"axon_start_nrt_profile rc={rc}")
                try:
                    yield
                finally:
                    n = lib.axon_stop_nrt_profile(str(output_dir).encode())
                    if n < 0:
                        raise RuntimeError(f"axon_stop_nrt_profile rc={n}")

        mod = types.ModuleType("antenv.axon_hooks")
        mod.get_axon_ntff_profile_hook = lambda: hook
        mod.set_axon_ntff_profile_hook = lambda h: None
        sys.modules["antenv.axon_hooks"] = mod
    except Exception:
        pass


_install_ntff_shim()

# ------------------------------------------------------------- host helpers


def _rope_tables(positions):
    dim = D_ROPE
    pos_freqs = BASE ** (np.arange(0, dim, 2, dtype=np.float64) / dim)
    inv_extra = 1.0 / pos_freqs
    inv_inter = 1.0 / (FACTOR * pos_freqs)

    def corr(nr):
        return dim * math.log(ORIG_MAX / (nr * 2 * math.pi)) / (2 * math.log(BASE))

    low = max(math.floor(corr(BETA_FAST)), 0)
    high = min(math.ceil(corr(BETA_SLOW)), dim - 1)
    ramp = np.clip(
        (np.arange(dim // 2, dtype=np.float64) - low) / max(high - low, 0.001), 0.0, 1.0
    )
    mask = 1.0 - ramp
    inv_freq = inv_inter * (1.0 - mask) + inv_extra * mask
    freqs = np.outer(np.asarray(positions, np.float64), inv_freq)
    m = _yarn_mscale(FACTOR, MSCALE) / _yarn_mscale(FACTOR, MSCALE_ALL)
    return (np.cos(freqs) * m).astype(np.float32), (np.sin(freqs) * m).astype(np.float32)


# ------------------------------------------------------------ bass builders
_BUILD_CACHE = {}


def _build_phase1():
    from concourse import bacc, mybir
    from concourse.tile import TileContext

    F32 = mybir.dt.float32
    BF16 = mybir.dt.bfloat16

    nc = bacc.Bacc()
    hTp = nc.dram_tensor("hTp", [128, KCH, TC1], BF16, kind="ExternalInput")
    wabp = nc.dram_tensor("wabp", [128, KCH, WABH], BF16, kind="ExternalInput")
    ab_out = nc.dram_tensor("ab", [128, TC1 // 128, WABH], BF16,
                            kind="ExternalOutput")

    MT = TC1 // 128  # 4 token tiles

    with TileContext(nc) as tc, contextlib.ExitStack() as ctx:
        pool = ctx.enter_context(tc.tile_pool(name="sb", bufs=1))

        hT_sb = pool.tile([128, KCH, TC1], BF16, tag="hT")
        wab_sb = pool.tile([128, KCH, WABH], BF16, tag="wab")
        ab_st = pool.tile([128, MT, WABH], BF16, tag="abst")
        warm_sb = pool.tile([128, 512], BF16, tag="warm")
        nc.vector.memset(warm_sb[:, :], 0.25)

        # warmup matmuls: no data deps, keep the PE busy through the DMA fill
        with tc.tile_pool(name="ppw", bufs=1, space="PSUM") as ppw:
            w_ps = ppw.tile([128, 512], F32, tag="w")
            for i in range(5):
                nc.tensor.matmul(w_ps[:, :], warm_sb[:, 0:128], warm_sb[:, :],
                                 start=True, stop=True)

        # loads: single ring, in consumption order; only 8 HWDGE completion
        # semaphores exist, so stay at <=8 outstanding dma_starts
        ksplits = [(0, 1), (1, 5), (5, 10), (10, 16)]
        for a, b in ksplits:
            nc.sync.dma_start(out=hT_sb[:, a:b, :], in_=hTp[:, a:b, :])
            nc.sync.dma_start(out=wab_sb[:, a:b, :], in_=wabp[:, a:b, :])

        # qa + kva chains interleaved per k: the static scheduler cannot fill
        # runtime DMA waits, so each k-chunk's full PE work is issued
        # together to ride the stream densely
        with tc.tile_pool(name="ppq", bufs=1, space="PSUM") as ppq, \
             tc.tile_pool(name="ppk", bufs=2, space="PSUM") as ppk:
            for mg in range(2):
                qa_ps = [ppq.tile([128, WQH], F32, tag=f"qa{mi}",
                                  name=f"qa{mg}_{mi}") for mi in range(2)]
                kv_ps = [ppk.tile([128, WKH], F32, tag=f"kv{mi}",
                                  name=f"kv{mg}_{mi}") for mi in range(2)]
                for k in range(KCH):
                    st = (k == 0)
                    sp = (k == KCH - 1)
                    for mi in range(2):
                        m = 2 * mg + mi
                        stat = hT_sb[:, k, m * 128:(m + 1) * 128]
                        nc.tensor.matmul(qa_ps[mi][:, 0:512], stat,
                                         wab_sb[:, k, 0:512],
                                         start=st, stop=sp)
                        nc.tensor.matmul(qa_ps[mi][:, 512:WQH], stat,
                                         wab_sb[:, k, 512:WQH],
                                         start=st, stop=sp)
                        nc.tensor.matmul(kv_ps[mi][:, :], stat,
                                         wab_sb[:, k, WQH:WABH],
                                         start=st, stop=sp)
                for mi in range(2):
                    m = 2 * mg + mi
                    if mi == 0:
                        nc.vector.tensor_copy(ab_st[:, m, 0:WQH],
                                              qa_ps[mi][:, :])
                        nc.scalar.copy(ab_st[:, m, WQH:WABH], kv_ps[mi][:, :])
                    else:
                        nc.scalar.copy(ab_st[:, m, 0:WQH], qa_ps[mi][:, :])
                        nc.vector.tensor_copy(ab_st[:, m, WQH:WABH],
                                              kv_ps[mi][:, :])
                eng = nc.sync if mg == 0 else nc.scalar
                eng.dma_start(out=ab_out[:, 2 * mg:2 * mg + 2, :],
                              in_=ab_st[:, 2 * mg:2 * mg + 2, :])

    nc.finalize()
    return nc


def _build_phase2():
    from concourse import bacc, mybir
    from concourse.tile import TileContext

    F32 = mybir.dt.float32
    BF16 = mybir.dt.bfloat16
    FP16 = mybir.dt.float16
    AF = mybir.ActivationFunctionType
    OP = mybir.AluOpType
    EXPB = -8.0 * math.log(2.0)  # exp bias; cancels in softmax, keeps fp16 range

    nc = bacc.Bacc()
    kvaTp = nc.dram_tensor("kvaTp", [128, KVCH, T], BF16, kind="ExternalInput")
    qaTp = nc.dram_tensor("qaTp", [128, RCH, T], BF16, kind="ExternalInput")
    kpe2 = nc.dram_tensor("kpe2", [128, T], BF16, kind="ExternalInput")
    wkbn = nc.dram_tensor("wkbn", [128, KVCH, HPC, 128], BF16, kind="ExternalInput")
    wkbv = nc.dram_tensor("wkbv", [128, KVCH, 256], BF16, kind="ExternalInput")
    wq3 = nc.dram_tensor("wq3", [128, RCH, 3, 128], BF16, kind="ExternalInput")
    wop = nc.dram_tensor("wop", [128, HPC, HID], BF16, kind="ExternalInput")
    csd = nc.dram_tensor("csd", [128, T], BF16, kind="ExternalInput")
    ssd = nc.dram_tensor("ssd", [128, T], BF16, kind="ExternalInput")
    maskd = nc.dram_tensor("maskd", [128, 896], FP16, kind="ExternalInput")
    out_p = nc.dram_tensor("out_p", [T, HID], BF16, kind="ExternalOutput")

    with TileContext(nc) as tc, contextlib.ExitStack() as ctx:
        persist = ctx.enter_context(tc.tile_pool(name="persist", bufs=1))

        kvaT_sb = persist.tile([128, KVCH, T], BF16, tag="kvaT")
        qaT_sb = persist.tile([128, RCH, T], BF16, tag="qaT")
        kpe2_sb = persist.tile([128, T], BF16, tag="kpe2")
        wkbn_sb = persist.tile([128, KVCH, HPC, 128], BF16, tag="wkbn")
        wkbv_sb = persist.tile([128, KVCH, 256], BF16, tag="wkbv")
        wq3_sb = persist.tile([128, RCH, 3, 128], BF16, tag="wq3")
        wo_sb = persist.tile([128, HPC, HID], BF16, tag="wo")
        cs_sb = persist.tile([128, T], BF16, tag="cs")
        ss_sb = persist.tile([128, T], BF16, tag="ss")
        mask_sb = persist.tile([128, 896], FP16, tag="mask")
        ones_sb = persist.tile([128, 128], FP16, tag="ones")
        nc.vector.memset(ones_sb[:, :], 1.0)
        expb_sb = persist.tile([128, 1], F32, tag="expb")
        nc.vector.memset(expb_sb[:, :], EXPB)
        warm_sb = persist.tile([128, 512], BF16, tag="warm")
        nc.vector.memset(warm_sb[:, :], 0.25)
        gate_sb = persist.tile([128, 2], BF16, tag="gate")

        knopeT = [persist.tile([128, T], BF16, tag=f"knopeT{h}", name=f"knopeT{h}") for h in range(HPC)]
        v_nat = [persist.tile([128, T], FP16, tag=f"vnat{h}", name=f"vnat{h}") for h in range(HPC)]
        qnT = [persist.tile([128, T], BF16, tag=f"qnT{h}", name=f"qnT{h}") for h in range(HPC)]
        qpeT = persist.tile([128, T], BF16, tag="qpeT")  # [h0 x'|y' ; h1 x'|y']
        aoT = [persist.tile([128, T], BF16, tag=f"aoT{h}", name=f"aoT{h}") for h in range(HPC)]
        colsum = [persist.tile([128, QT], FP16, tag=f"colsum{h}", name=f"colsum{h}") for h in range(HPC)]

        # warmup matmuls bridge the runtime preamble + first-load latency
        with tc.tile_pool(name="ppw", bufs=1, space="PSUM") as ppw:
            w_ps = ppw.tile([128, 512], F32, tag="w")
            for i in range(9):
                nc.tensor.matmul(w_ps[:, :], warm_sb[:, 0:128], warm_sb[:, :],
                                 start=True, stop=True)

        # loads, split across the three DMA-issue engines:
        #  SP ring: PE-critical order (stage A/B inputs), streams alone first
        #  Act ring: the big qaT stream, gated behind the critical kva loads
        #  gpsimd SWDGE: late attention inputs, same gate
        # single HWDGE ring, strict consumption order: ring order IS the
        # execution order, which encodes criticality without gate hacks; big
        # consolidated loads keep the 8 completion semaphores from running
        # out (they recycle as the consuming matmuls execute)
        nc.sync.dma_start(out=wkbn_sb[:, :, :, :], in_=wkbn[:, :, :, :])
        nc.sync.dma_start(out=kvaT_sb[:, 0:1, :], in_=kvaTp[:, 0:1, :])
        nc.sync.dma_start(out=kvaT_sb[:, 1:2, :], in_=kvaTp[:, 1:2, :])
        nc.sync.dma_start(out=kvaT_sb[:, 2:4, :], in_=kvaTp[:, 2:4, :])
        nc.sync.dma_start(out=wq3_sb[:, 0:6, :, :], in_=wq3[:, 0:6, :, :])
        nc.sync.dma_start(out=qaT_sb[:, 0:2, :], in_=qaTp[:, 0:2, :])
        nc.sync.dma_start(out=wq3_sb[:, 6:12, :, :], in_=wq3[:, 6:12, :, :])
        nc.sync.dma_start(out=qaT_sb[:, 2:5, :], in_=qaTp[:, 2:5, :])
        nc.sync.dma_start(out=wkbv_sb[:, :, :], in_=wkbv[:, :, :])
        nc.sync.dma_start(out=qaT_sb[:, 5:8, :], in_=qaTp[:, 5:8, :])
        nc.sync.dma_start(out=cs_sb[:, :], in_=csd[:, :])
        nc.sync.dma_start(out=ss_sb[:, :], in_=ssd[:, :])
        nc.sync.dma_start(out=qaT_sb[:, 8:12, :], in_=qaTp[:, 8:12, :])
        nc.sync.dma_start(out=kpe2_sb[:, :], in_=kpe2[:, :])
        nc.sync.dma_start(out=mask_sb[:, :], in_=maskd[:, :])
        nc.sync.dma_start(out=wo_sb[:, :, :], in_=wop[:, :, :])

        # ---------------- stage 1a: k_nope^T = wkbn^T kva, k-outer ----------
        with tc.tile_pool(name="ppkn", bufs=1, space="PSUM") as ppkn:
            kn_ps = [[ppkn.tile([128, 512], F32, tag=f"kn{h}_{n}", name=f"kn{h}_{n}")
                      for n in range(4)] for h in range(HPC)]
            for k in range(KVCH):
                for h in range(HPC):
                    for n in range(4):
                        nsl = slice(n * 512, (n + 1) * 512)
                        nc.tensor.matmul(
                            kn_ps[h][n][:, :], wkbn_sb[:, k, h, :],
                            kvaT_sb[:, k, nsl],
                            start=(k == 0), stop=(k == KVCH - 1))
            for h in range(HPC):
                for n in range(4):
                    nsl = slice(n * 512, (n + 1) * 512)
                    if n % 2 == 0:
                        nc.scalar.copy(knopeT[h][:, nsl], kn_ps[h][n][:, :])
                    else:
                        nc.vector.tensor_copy(knopeT[h][:, nsl],
                                              kn_ps[h][n][:, :])

        # ------------- stage 2: q up-projections + rope (2-qtr groups);
        # v up-projection shares the PSUM scope (disjoint banks) so there is
        # no pool barrier before it and it bridges the last rope tail
        with tc.tile_pool(name="ppg", bufs=2, space="PSUM") as ppg, \
             tc.tile_pool(name="ppv", bufs=2, space="PSUM") as ppv, \
             tc.tile_pool(name="qwork", bufs=2) as qwork:
            for qg in range(2):
                qtrs = [2 * qg, 2 * qg + 1]
                qn_ps = {}
                qp_ps = {}
                for qtr in qtrs:
                    qn_ps[qtr] = [ppg.tile([128, QT], F32, tag=f"qn{h}",
                                           name=f"qn{h}_{qtr}")
                                  for h in range(HPC)]
                    qp_ps[qtr] = ppg.tile([128, QT], F32, tag="qp",
                                          name=f"qp_{qtr}")
                for k in range(RCH):
                    st = (k == 0)
                    sp = (k == RCH - 1)
                    # stationary-reuse order: each stationary serves both qtrs
                    for h in range(HPC):
                        for qtr in qtrs:
                            qsl = slice(qtr * QT, (qtr + 1) * QT)
                            nc.tensor.matmul(qn_ps[qtr][h][:, :],
                                             wq3_sb[:, k, h, :],
                                             qaT_sb[:, k, qsl],
                                             start=st, stop=sp)
                    for qtr in qtrs:
                        qsl = slice(qtr * QT, (qtr + 1) * QT)
                        nc.tensor.matmul(qp_ps[qtr][:, :], wq3_sb[:, k, 2, :],
                                         qaT_sb[:, k, qsl],
                                         start=st, stop=sp)
                for qtr in qtrs:
                    qsl = slice(qtr * QT, (qtr + 1) * QT)
                    # rope first: it is the longest chain (copy -> swaps ->
                    # DVE muls) gating the attention start; qnT copies follow
                    qp_sb = qwork.tile([128, QT], F32, tag="qpsb",
                                       name=f"qpsb{qtr}")
                    nc.scalar.copy(qp_sb[:, :], qp_ps[qtr][:, :])
                    sw_sb = qwork.tile([128, QT], F32, tag="swsb",
                                       name=f"swsb{qtr}")
                    nc.gpsimd.dma_start(out=sw_sb[0:32, :], in_=qp_sb[32:64, :])
                    nc.gpsimd.dma_start(out=sw_sb[32:64, :], in_=qp_sb[0:32, :])
                    nc.gpsimd.dma_start(out=sw_sb[64:96, :], in_=qp_sb[96:128, :])
                    nc.gpsimd.dma_start(out=sw_sb[96:128, :], in_=qp_sb[64:96, :])
                    ta = qwork.tile([128, QT], BF16, tag="ta", name=f"ta{qtr}")
                    nc.vector.tensor_tensor(ta[:, :], qp_sb[:, :], cs_sb[:, qsl],
                                            op=OP.mult)
                    tb = qwork.tile([128, QT], BF16, tag="tb", name=f"tb{qtr}")
                    nc.vector.tensor_tensor(tb[:, :], sw_sb[:, :], ss_sb[:, qsl],
                                            op=OP.mult)
                    nc.vector.tensor_tensor(qpeT[:, qsl], ta[:, :], tb[:, :],
                                            op=OP.add)
                    nc.scalar.copy(qnT[0][:, qsl], qn_ps[qtr][0][:, :])
                    nc.vector.tensor_copy(qnT[1][:, qsl], qn_ps[qtr][1][:, :])

            # v up-projection: real PE work independent of the rope tables
            for t in range(T // 128):
                tsl = slice(t * 128, (t + 1) * 128)
                v_ps = ppv.tile([128, 256], F32, tag="v", name=f"v{t}")
                for k in range(KVCH):
                    nc.tensor.matmul(v_ps[:, :], kvaT_sb[:, k, tsl],
                                     wkbv_sb[:, k, :],
                                     start=(k == 0), stop=(k == KVCH - 1))
                if t % 2 == 0:
                    nc.scalar.copy(v_nat[0][:, tsl], v_ps[:, 0:128])
                    nc.vector.tensor_copy(v_nat[1][:, tsl], v_ps[:, 128:256])
                else:
                    nc.vector.tensor_copy(v_nat[0][:, tsl], v_ps[:, 0:128])
                    nc.scalar.copy(v_nat[1][:, tsl], v_ps[:, 128:256])

        # ------- stage 3: attention; softmax tail + o_proj pipelined 1 qtr ----
        # both heads of a k-tile share one [128, 2, 512] PSUM pair so the
        # exp / mask / colsum work runs as ONE wide instruction per k-tile
        # (instruction overhead halved) and the four score matmuls become
        # ready together -> the two 64-row rope matmuls issue adjacently and
        # run concurrently in disjoint row-groups of the PE array
        with tc.tile_pool(name="pps", bufs=2, space="PSUM") as pps, \
             tc.tile_pool(name="ppu", bufs=2, space="PSUM") as ppu, \
             tc.tile_pool(name="ppo", bufs=2, space="PSUM") as ppo, \
             tc.tile_pool(name="awork", bufs=8) as awork, \
             tc.tile_pool(name="rwork", bufs=2) as rwork, \
             tc.tile_pool(name="owork", bufs=3) as owork:

            o_sb_map = {}
            o_count = [0]

            def emit_o_unit(qtr, tt, j):
                # one o_proj 512-col chunk for token tile tt of q-range qtr
                q0o = qtr * QT
                tslo = slice(q0o + tt * 128, q0o + (tt + 1) * 128)
                jsl = slice(j * 512, (j + 1) * 512)
                if j == 0:
                    o_sb_map[(qtr, tt)] = owork.tile(
                        [128, HID], BF16, tag="osb", name=f"o{qtr}_{tt}")
                o_sb = o_sb_map[(qtr, tt)]
                o_ps = ppo.tile([128, 512], F32, tag="o",
                                name=f"op{qtr}_{tt}_{j}")
                for h in range(HPC):
                    nc.tensor.matmul(o_ps[:, :], aoT[h][:, tslo],
                                     wo_sb[:, h, jsl],
                                     start=(h == 0), stop=(h == HPC - 1))
                o_count[0] += 1
                if o_count[0] % 2 == 0:
                    nc.vector.tensor_copy(o_sb[:, jsl], o_ps[:, :])
                else:
                    nc.scalar.copy(o_sb[:, jsl], o_ps[:, :])
                if j == 3:
                    nc.sync.dma_start(out=out_p[tslo, :], in_=o_sb[:, :])

            def make_tail(qtr, h, un_t):
                def emit():
                    qslh = slice(qtr * QT, (qtr + 1) * QT)
                    denb_ps = ppo.tile([128, QT], F32, tag="o",
                                       name=f"db{h}_{qtr}")
                    nc.tensor.matmul(denb_ps[:, :], ones_sb[:, :],
                                     colsum[h][:, :], start=True, stop=True,
                                     skip_group_check=True)
                    recip = rwork.tile([128, QT], F32, tag="recip",
                                       name=f"r{h}_{qtr}")
                    nc.vector.reciprocal_approx_fast(out=recip[:, :],
                                                     in_=denb_ps[:, :])
                    nc.vector.tensor_tensor(aoT[h][:, qslh], un_t[h][:, :],
                                            recip[:, :], op=OP.mult)
                return emit

            filler = []
            # causally independent q-ranges; end on the shortest (qtr 0) so
            # the previous qtr's deferred tail drains inside a loop, leaving
            # the smallest possible serial flush
            for qtr in (1, 2, 3, 0):
                q0 = qtr * QT
                n_k = (q0 + QT) // KT
                # softmax tails of the previous qtr run first: they free the
                # un tiles this qtr's AV accumulation needs
                for _ in range(2):
                    if filler:
                        filler.pop(0)()
                un_ps = [ppu.tile([128, QT], F32, tag="un",
                                  name=f"un{h}_{qtr}") for h in range(HPC)]
                for ki in range(n_k):
                    k0 = ki * KT
                    ksl = slice(k0, k0 + KT)
                    d = k0 - q0
                    coff = max(d, 0)
                    w = QT - coff
                    msl = slice(q0 + coff, q0 + QT)
                    if filler:
                        filler.pop(0)()
                    # both heads' scores share one [128, 2, 512] PSUM pair:
                    # the four matmuls become ready together, so the two
                    # 64-row rope matmuls issue adjacently and run
                    # concurrently in disjoint row-groups of the PE array
                    s2 = pps.tile([128, HPC, QT], F32, tag="s",
                                  name=f"s_{qtr}_{ki}")
                    for h in range(HPC):
                        nc.tensor.matmul(s2[:, h, 0:w], knopeT[h][:, ksl],
                                         qnT[h][:, msl], start=True, stop=False,
                                         skip_group_check=True)
                    for h in range(HPC):
                        hb = slice(64 * h, 64 * h + 64)
                        nc.tensor.matmul(s2[:, h, 0:w], kpe2_sb[hb, ksl],
                                         qpeT[hb, msl], start=False, stop=True,
                                         skip_group_check=True)
                    if filler:
                        filler.pop(0)()
                    # per-head softmax plumbing on contiguous 2D slices
                    # (strided 3D APs run far below peak on ACT/DVE)
                    for h in range(HPC):
                        if ki == 0:
                            # exp straight into the colsum tile: saves a copy
                            nc.scalar.activation(out=colsum[h][:, 0:w],
                                                 in_=s2[:, h, 0:w],
                                                 func=AF.Exp, scale=SCALING,
                                                 bias=expb_sb[:, :])
                            if d >= 0:
                                nc.vector.tensor_tensor(
                                    colsum[h][:, 0:w], colsum[h][:, 0:w],
                                    mask_sb[:, 384:384 + w], op=OP.mult)
                            mov = colsum[h]
                        else:
                            expT = awork.tile([128, QT], FP16, tag="expT",
                                              name=f"e{h}_{qtr}_{ki}")
                            nc.scalar.activation(out=expT[:, 0:w],
                                                 in_=s2[:, h, 0:w],
                                                 func=AF.Exp, scale=SCALING,
                                                 bias=expb_sb[:, :])
                            if d >= 0:
                                nc.vector.tensor_tensor(
                                    expT[:, 0:w], expT[:, 0:w],
                                    mask_sb[:, 384:384 + w], op=OP.mult)
                            nc.vector.tensor_tensor(
                                colsum[h][:, coff:QT], colsum[h][:, coff:QT],
                                expT[:, 0:w], op=OP.add)
                            mov = expT
                        nc.tensor.matmul(un_ps[h][:, coff:QT],
                                         v_nat[h][:, ksl], mov[:, 0:w],
                                         start=(ki == 0), stop=(ki == n_k - 1),
                                         skip_group_check=True)
                while filler:
                    filler.pop(0)()
                filler = [make_tail(qtr, h, un_ps) for h in range(HPC)]
                filler += [(lambda a, b, c: (lambda: emit_o_unit(a, b, c)))(
                    qtr, tt, j) for tt in range(QT // 128) for j in range(4)]
            while filler:
                filler.pop(0)()

    nc.finalize()
    return nc


def _get_built(name):
    if name not in _BUILD_CACHE:
        _BUILD_CACHE[name] = _build_phase1() if name == "p1" else _build_phase2()
    return _BUILD_CACHE[name]


# ---------------------------------------------------------------- kernel()


def kernel(positions, hidden_states, wq_a, q_a_norm_w, wq_b, wkv_a, kv_a_norm_w,
           wkv_b, wo):
    import os

    from concourse.bass_utils import run_bass_kernel_spmd
    import ml_dtypes

    BFNP = ml_dtypes.bfloat16
    trace = bool(os.environ.get("BASS_KERNEL_TRACE"))
    LAST_EXEC_NS.clear()

    positions = np.asarray(positions)
    hidden = np.asarray(hidden_states, np.float32)
    wq_a = np.asarray(wq_a, np.float32)
    wq_b = np.asarray(wq_b, np.float32)
    wkv_a = np.asarray(wkv_a, np.float32)
    wkv_b = np.asarray(wkv_b, np.float32)
    wo = np.asarray(wo, np.float32)
    q_a_norm_w = np.asarray(q_a_norm_w, np.float32)
    kv_a_norm_w = np.asarray(kv_a_norm_w, np.float32)

    cos, sin = _rope_tables(positions)  # [T, 32] f32

    # ---------------- launch 1: latents (4 seq blocks x 2 col halves) -------
    hidden_bf = hidden.astype(BFNP)
    wab_halves = []
    for half in range(2):
        wq_h = wq_a[:, half * WQH:(half + 1) * WQH]
        wk_h = wkv_a[:, half * WKH:(half + 1) * WKH]
        wab = np.concatenate([wq_h, wk_h], axis=1).astype(BFNP)  # [2048, 1056]
        wab_halves.append(np.ascontiguousarray(
            wab.reshape(KCH, 128, WABH).transpose(1, 0, 2)))

    in_maps1 = []
    for c in range(N_CORES):
        seq, half = c // 2, c % 2
        hs = hidden_bf[seq * TC1:(seq + 1) * TC1]  # [512, 2048]
        hTp = np.ascontiguousarray(hs.reshape(TC1, KCH, 128).transpose(2, 1, 0))
        in_maps1.append({
            "hTp": hTp,
            "wabp": wab_halves[half],
        })

    nc1 = _get_built("p1")
    res1 = run_bass_kernel_spmd(nc1, in_maps1, core_ids=list(range(N_CORES)),
                                trace=trace)
    if trace:
        LAST_EXEC_NS.append(res1.exec_time_ns)

    qa_full = np.empty((T, Q_RANK), np.float32)
    kva_full = np.empty((T, KV_RANK + D_ROPE), np.float32)
    for c in range(N_CORES):
        seq, half = c // 2, c % 2
        r = res1.results[c]
        tsl = slice(seq * TC1, (seq + 1) * TC1)
        ab = np.asarray(r["ab"], np.float32).transpose(1, 0, 2).reshape(
            TC1, WABH)
        qa_full[tsl, half * WQH:(half + 1) * WQH] = ab[:, 0:WQH]
        kva_full[tsl, half * WKH:(half + 1) * WKH] = ab[:, WQH:WABH]

    # host RMSNorm (ssq from the bf16 latents; negligible vs fp32)
    rstd_q = 1.0 / np.sqrt((qa_full ** 2).mean(1) + EPS)
    rstd_kv = 1.0 / np.sqrt((kva_full[:, :KV_RANK] ** 2).mean(1) + EPS)

    qa_n = qa_full * rstd_q[:, None]
    kva_n = kva_full[:, :KV_RANK] * rstd_kv[:, None]
    kpe_raw = kva_full[:, KV_RANK:]  # [T, 64], not normalized

    # host rope for k_pe (shared across heads); rows de-interleaved evens|odds
    x1, x2 = kpe_raw[:, 0::2], kpe_raw[:, 1::2]
    kx = x1 * cos - x2 * sin
    ky = x2 * cos + x1 * sin
    kpeT = np.concatenate([kx.T, ky.T], 0)          # [64, T]
    kpe2_host = np.ascontiguousarray(
        np.concatenate([kpeT, kpeT], 0).astype(BFNP))  # [128, T]

    qaT_p = np.ascontiguousarray(
        qa_n.T.reshape(RCH, 128, T).transpose(1, 0, 2).astype(BFNP))
    kvaT_p = np.ascontiguousarray(
        kva_n.T.reshape(KVCH, 128, T).transpose(1, 0, 2).astype(BFNP))

    # q-rope tables: CS = cos tiled 4x, SS = [-s, +s, -s, +s]
    cos_t = cos.T  # [32, T]
    sin_t = sin.T
    cs_host = np.ascontiguousarray(
        np.concatenate([cos_t] * 4, 0).astype(BFNP))
    ss_host = np.ascontiguousarray(
        np.concatenate([-sin_t, sin_t, -sin_t, sin_t], 0).astype(BFNP))

    cols = np.arange(896) - 384
    bigmask = np.ascontiguousarray(
        (cols[None, :] >= np.arange(128)[:, None]).astype(np.float16))

    # fold RMSNorm weights into the up-projection weights
    wq_b_eff = (wq_b * q_a_norm_w[:, None]).reshape(Q_RANK, H, D_QK)
    wkv_b_eff = (wkv_b * kv_a_norm_w[:, None]).reshape(KV_RANK, H, D_NOPE + D_V)
    wo_r = wo.reshape(H, D_V, HID)

    def pack_stat(w):  # [K*128, M] -> [128, K, M]
        kch = w.shape[0] // 128
        return np.ascontiguousarray(
            w.reshape(kch, 128, w.shape[1]).transpose(1, 0, 2).astype(BFNP))

    # ---------------- launch 2 ----------------
    nc2 = _get_built("p2")
    in_maps2 = []
    for c in range(N_CORES):
        heads = [c * HPC, c * HPC + 1]
        wqbn_w = wq_b_eff[:, heads, :D_NOPE].reshape(Q_RANK, HPC * 128)
        pe = wq_b_eff[:, heads, D_NOPE:]            # [R, 2, 64]
        px, py = pe[:, :, 0::2], pe[:, :, 1::2]     # [R, 2, 32]
        wqbp_w = np.concatenate(
            [px[:, 0], py[:, 0], px[:, 1], py[:, 1]], 1)  # [R, 128] order C
        wq3_w = np.concatenate([wqbn_w, wqbp_w], 1)  # [R, 384]
        wkbn_w = wkv_b_eff[:, heads, :D_NOPE].reshape(KV_RANK, HPC * 128)
        wkbv_w = wkv_b_eff[:, heads, D_NOPE:].reshape(KV_RANK, HPC * 128)
        in_maps2.append({
            "kvaTp": kvaT_p,
            "qaTp": qaT_p,
            "kpe2": kpe2_host,
            "wkbn": np.ascontiguousarray(
                pack_stat(wkbn_w).reshape(128, KVCH, HPC, 128)),
            "wkbv": pack_stat(wkbv_w),
            "wq3": np.ascontiguousarray(
                pack_stat(wq3_w).reshape(128, RCH, 3, 128)),
            "wop": np.ascontiguousarray(
                wo_r[heads].transpose(1, 0, 2).astype(BFNP)),
            "csd": cs_host,
            "ssd": ss_host,
            "maskd": bigmask,
        })
    res2 = run_bass_kernel_spmd(nc2, in_maps2, core_ids=list(range(N_CORES)),
                                trace=trace)
    if trace:
        LAST_EXEC_NS.append(res2.exec_time_ns)

    out = np.zeros((T, HID), np.float64)
    for c in range(N_CORES):
        out += res2.results[c]["out_p"].astype(np.float64)
    return out.astype(np.float32)
